# revision 27
# baseline (speedup 1.0000x reference)
"""HSTU block kernel for 8 TRN2 NeuronCores (Bass/Tile, bf16 matmuls).

Sharding: phase 1 (f1 + attention + u-gating) is data-parallel over batch
(B=2) x tensor-parallel over head groups (4 heads/core). Phase 2
(ln1 -> f2 -> +residual -> ln2) is row-parallel (512 rows/core). Host
gathers/reshards between the two launches; everything on-device is
feature-major so no transposes are ever needed.

v2: bf16 data path, causal mask folded into the pos-bias strip (-60 at
masked entries, silu(-60) == 0), attention bias adds on DVE instead of
PE, head-pair tiles via matmul tile_position, and a software-pipelined
emit order (qk block g interleaved with av block g-1, f1 slab g+1
between them) so the PE never head-of-line blocks on scalar silu.
"""
import os
import numpy as np
import ml_dtypes

import concourse.bacc as bacc
import concourse.mybir as mybir
from concourse.tile import TileContext
from concourse.bass_utils import run_bass_kernel_spmd

fp32 = mybir.dt.float32
bf16 = mybir.dt.bfloat16
AF = mybir.ActivationFunctionType
ALU = mybir.AluOpType

B, S, D, H, M = 2, 2048, 1024, 16, 4096
HD = D // H          # 64
EPS = 1e-5
P = 128
NB = S // P          # 16 seq blocks of 128
NG = S // 512        # 4 q-groups of 512
DC = 4 * HD          # 256 features per core in phase 1
NEG = -60.0          # silu(-60) == 0 in fp32: masked-score sentinel

_CACHE = {}

bfdt = ml_dtypes.bfloat16


# ---------------------------------------------------------------- kernel A
def build_kernel_a(units, jmin, n_strip, pair_jjs):
    """units[g] = [("pair", kt)] / [("single", kt, off, end)] per q-group.

    pair units cover (kt, kt+1), both with full [0,512) live range; their
    bias windows are pre-concatenated in strip2 indexed by jj0=4g-kt+15.
    """
    nc = bacc.Bacc("TRN2", target_bir_lowering=False, debug=False, num_devices=8)

    xT = nc.dram_tensor("xT", [D, S], bf16, kind="ExternalInput")
    w1T_qku = nc.dram_tensor("w1T_qku", [D, 768], bf16, kind="ExternalInput")
    w1T_v = nc.dram_tensor("w1T_v", [D, DC], bf16, kind="ExternalInput")
    b1_2d = nc.dram_tensor("b1_2d", [P, 6], fp32, kind="ExternalInput")
    b1v2 = nc.dram_tensor("b1v2", [P, 512], bf16, kind="ExternalInput")
    inv128 = nc.dram_tensor("inv128", [P, P], bf16, kind="ExternalInput")
    strip = nc.dram_tensor("strip", [P, n_strip * P], bf16, kind="ExternalInput")
    n_pj = max(len(pair_jjs), 1)
    strip2 = nc.dram_tensor("strip2", [P, n_pj * 1024], bf16,
                            kind="ExternalInput")
    yT_out = nc.dram_tensor("yT_out", [DC, S], bf16, kind="ExternalOutput")
    pj_idx = {jj: i for i, jj in enumerate(pair_jjs)}

    with TileContext(nc) as tc:
        with tc.tile_pool(name="const", bufs=1) as cpool, \
             tc.tile_pool(name="wpool", bufs=2) as wpool, \
             tc.tile_pool(name="big", bufs=1) as big, \
             tc.tile_pool(name="att", bufs=1) as apool, \
             tc.tile_pool(name="out", bufs=2) as opool, \
             tc.tile_pool(name="ps", bufs=1, space="PSUM") as ps:

            # HAM warmup: dense zero-matmuls (full 128-row contraction) keep
            # the PE activity monitor busy during the initial DMA wait so
            # the clock gate is released before real matmuls start
            wzero = cpool.tile([P, 512], bf16, name="wzero")
            nc.vector.memset(wzero[:], 0.0)
            for i in range(30):
                wp = ps.tile([P, 512], fp32, name="warm", tag="mm", bufs=2)
                nc.tensor.matmul(wp[:], wzero[:, 0:P], wzero[:],
                                 start=True, stop=True)

            # critical path: x slab 0 (sync queue) and w1 (scalar queue)
            # land in parallel so the first matmul starts early
            xs0 = []
            wq = []
            for k in range(8):
                t = wpool.tile([P, 512], bf16, name="xs", tag=f"xs{k}", bufs=2)
                nc.sync.dma_start(t[:], xT[k * P:(k + 1) * P, 0:512])
                xs0.append(t)
                w = big.tile([P, 768], bf16, name=f"wq{k}", tag=f"wq{k}")
                nc.scalar.dma_start(w[:], w1T_qku[k * P:(k + 1) * P, :])
                wq.append(w)
            b1_sb = cpool.tile([P, 6], fp32, name="b1_sb")
            nc.scalar.dma_start(b1_sb[:], b1_2d[:])
            strip_sb = big.tile([P, n_strip * P], bf16, name="strip_sb",
                                tag="strip")
            nc.sync.dma_start(strip_sb[:], strip[:])
            strip2_sb = big.tile([P, n_pj * 1024], bf16, name="strip2_sb",
                                 tag="strip2")
            nc.sync.dma_start(strip2_sb[:], strip2[:])
            wv = []
            for k in range(8):
                t = big.tile([P, DC], bf16, name=f"w1v{k}", tag=f"w1v{k}")
                nc.sync.dma_start(t[:], w1T_v[k * P:(k + 1) * P, :])
                wv.append(t)
            b1v_sb = cpool.tile([P, 512], bf16, name="b1v_sb")
            nc.sync.dma_start(b1v_sb[:], b1v2[:])
            inv128_sb = cpool.tile([P, P], bf16, name="inv128_sb")
            nc.sync.dma_start(inv128_sb[:], inv128[:])

            # persistent: k heads (pair tiles) + v (sc-pair tiles)
            kh = [big.tile([P, S], bf16, name=f"khp{p}", tag=f"khp{p}")
                  for p in range(2)]
            v_sb = [big.tile([P, 512], bf16, name=f"v{scp}", tag=f"v{scp}")
                    for scp in range(8)]

            def f1_slab(sg, xs, first=False):
                """f1 for seq slab sg: q/k/u (feature-major pair tiles) + v."""
                sl = slice(sg * 512, (sg + 1) * 512)
                qs = [wpool.tile([P, 512], bf16, name="qs", tag=f"qs{i}",
                                 bufs=2) for i in range(2)]
                us = [wpool.tile([P, 512], bf16, name="us", tag=f"us{i}",
                                 bufs=2) for i in range(2)]

                def act_fc(fc, pt):
                    bcol = b1_sb[:, fc:fc + 1]
                    if fc < 2:
                        nc.scalar.activation(qs[fc][:], pt[:], AF.Silu,
                                             bias=bcol, scale=1.0)
                    elif fc < 4:
                        nc.scalar.activation(kh[fc - 2][:, sl], pt[:],
                                             AF.Silu, bias=bcol, scale=1.0)
                    else:
                        nc.scalar.activation(us[fc - 4][:], pt[:], AF.Silu,
                                             bias=bcol, scale=1.0)

                if first:
                    # k-outer in fc pairs so compute starts on chunk 0
                    for fc0 in (0, 2):
                        pts = [ps.tile([P, 512], fp32, name="f1ps", tag="mm",
                                       bufs=2) for _ in range(2)]
                        for k in range(8):
                            for j in range(2):
                                fc = fc0 + j
                                nc.tensor.matmul(
                                    pts[j][:], wq[k][:, fc * P:(fc + 1) * P],
                                    xs[k][:], start=(k == 0), stop=(k == 7))
                        for j in range(2):
                            act_fc(fc0 + j, pts[j])
                    fcs_rest = (4, 5)
                else:
                    fcs_rest = range(6)
                for fc in fcs_rest:
                    pt = ps.tile([P, 512], fp32, name="f1ps", tag="mm",
                                 bufs=2)
                    for k in range(8):
                        nc.tensor.matmul(pt[:], wq[k][:, fc * P:(fc + 1) * P],
                                         xs[k][:],
                                         start=(k == 0), stop=(k == 7))
                    act_fc(fc, pt)
                # v: natural (seq-major) layout, 2 sc per psum tile
                for scp in (2 * sg, 2 * sg + 1):
                    pt = ps.tile([P, 512], fp32, name="f1vps", tag="mm",
                                 bufs=2)
                    for c in range(2):
                        csl = slice(c * DC, (c + 1) * DC)
                        xoff = (2 * (scp - 2 * sg) + c) * P
                        for k in range(8):
                            nc.tensor.matmul(
                                pt[:, csl], xs[k][:, xoff:xoff + P], wv[k][:],
                                start=(k == 0), stop=False)
                        nc.tensor.matmul(pt[:, csl], inv128_sb[:],
                                         b1v_sb[:, csl],
                                         start=False, stop=True)
                    nc.scalar.activation(v_sb[scp][:], pt[:], AF.Silu,
                                         scale=1.0)
                return qs, us

            state = {}  # per g: (qs, us, att tiles, avp psums)

            def emit_qk(g, p, ui, unit):
                """scores + bias + silu for head pair p, one unit."""
                qs = state[g]["qs"]
                if unit[0] == "pair":
                    kt = unit[1]
                    jj0 = 4 * g - kt + 15
                    s2 = pj_idx[jj0] * 1024
                    for hp in range(2):
                        h0 = hp * 64
                        spp = ps.tile([P, 1024], fp32, name="sp2", tag="sp2",
                                      bufs=2)
                        for c in range(2):
                            nc.tensor.matmul(
                                spp[:, c * 512:(c + 1) * 512],
                                kh[p][h0:h0 + 64,
                                      (kt + c) * P:(kt + c + 1) * P],
                                qs[p][h0:h0 + 64, :],
                                start=True, stop=True)
                        att = apool.tile([P, 1024], bf16, name="att2",
                                         tag=f"a2_{p}{hp}u{ui}", bufs=1)
                        nc.vector.tensor_tensor(
                            att[:], spp[:], strip2_sb[:, s2:s2 + 1024],
                            ALU.add)
                        nc.scalar.activation(att[:], att[:], AF.Silu,
                                             scale=1.0)
                        state[g]["att"][(p, hp, ui)] = att
                else:
                    _, kt, off, end = unit
                    sb0 = (4 * g - kt + 15 - jmin) * P
                    for hp in range(2):
                        h0 = hp * 64
                        spp = ps.tile([P, 512], fp32, name="sps", tag="mm",
                                      bufs=2)
                        nc.tensor.matmul(
                            spp[:, off:end],
                            kh[p][h0:h0 + 64, kt * P:(kt + 1) * P],
                            qs[p][h0:h0 + 64, off:end],
                            start=True, stop=True)
                        att = apool.tile([P, 512], bf16, name="att",
                                         tag=f"a{p}{hp}u{ui}", bufs=1)
                        nc.vector.tensor_tensor(
                            att[:, off:end], spp[:, off:end],
                            strip_sb[:, sb0 + off:sb0 + end], ALU.add)
                        nc.scalar.activation(att[:, off:end],
                                             att[:, off:end],
                                             AF.Silu, scale=1.0)
                        state[g]["att"][(p, hp, ui)] = att

            def emit_av(g, p, ui, unit, n_units):
                """accumulation steps of att @ v for head pair p, one unit."""
                if ui == 0:
                    state[g]["avp"][p] = ps.tile([P, 512], fp32, name="avp",
                                                 tag=f"avp{p}", bufs=1)
                avp = state[g]["avp"][p]
                att_d = state[g]["att"]
                last = (ui == n_units - 1)
                if unit[0] == "pair":
                    kt0 = unit[1]
                    for hp in range(2):
                        h = 2 * p + hp
                        att = att_d[(p, hp, ui)]
                        for c in range(2):
                            kt = kt0 + c
                            scp, cc = kt // 2, kt % 2
                            nc.tensor.matmul(
                                avp[hp * 64:(hp + 1) * 64, :],
                                v_sb[scp][:, cc * DC + h * 64:
                                          cc * DC + (h + 1) * 64],
                                att[:, c * 512:(c + 1) * 512],
                                start=(ui == 0 and c == 0),
                                stop=(last and c == 1))
                else:
                    _, kt, off, end = unit
                    scp, cc = kt // 2, kt % 2
                    ao, ae = (0, 512) if ui == 0 else (off, end)
                    for hp in range(2):
                        h = 2 * p + hp
                        att = att_d[(p, hp, ui)]
                        nc.tensor.matmul(
                            avp[hp * 64:(hp + 1) * 64, ao:ae],
                            v_sb[scp][:, cc * DC + h * 64:
                                      cc * DC + (h + 1) * 64],
                            att[:, ao:ae],
                            start=(ui == 0), stop=last)

            def emit_yg(g):
                """u-gating + output DMA for group g (both pairs)."""
                us = state[g]["us"]
                for p in range(2):
                    avp = state[g]["avp"][p]
                    yg = opool.tile([P, 512], bf16, name="yg", tag=f"yg{p}",
                                    bufs=2)
                    nc.vector.tensor_tensor(yg[:], avp[:], us[p][:], ALU.mult)
                    nc.sync.dma_start(
                        yT_out[p * P:(p + 1) * P, g * 512:(g + 1) * 512],
                        yg[:])
                del state[g]

            def block(g):
                """qk stream for g interleaved with av stream for g-1."""
                # prefetch x slab g+1
                if g < NG - 1:
                    xs_n = []
                    nsl = slice((g + 1) * 512, (g + 2) * 512)
                    for k in range(8):
                        t = wpool.tile([P, 512], bf16, name="xs",
                                       tag=f"xs{k}", bufs=2)
                        nc.sync.dma_start(t[:], xT[k * P:(k + 1) * P, nsl])
                        xs_n.append(t)
                else:
                    xs_n = None
                prev = units[g - 1] if g > 0 else []
                cur = units[g]
                for p in range(2):
                    for ui, unit in enumerate(cur):
                        if ui < len(prev):
                            emit_av(g - 1, p, ui, prev[ui], len(prev))
                        emit_qk(g, p, ui, unit)
                if g > 0:
                    emit_yg(g - 1)
                return xs_n

            qs, us = f1_slab(0, xs0, first=True)
            state[0] = {"qs": qs, "us": us, "att": {}, "avp": {}}
            xs_next = block(0)
            for g in range(1, NG):
                qs, us = f1_slab(g, xs_next)
                state[g] = {"qs": qs, "us": us, "att": {}, "avp": {}}
                xs_next = block(g)
            # drain: av + yg for the last group
            gl = NG - 1
            cur = units[gl]
            for p in range(2):
                for ui, unit in enumerate(cur):
                    emit_av(gl, p, ui, unit, len(cur))
            emit_yg(gl)
    return nc


# ---------------------------------------------------------------- kernel B
def build_kernel_b():
    nc = bacc.Bacc("TRN2", target_bir_lowering=False, debug=False, num_devices=8)

    yT = nc.dram_tensor("yT", [D, 512], bf16, kind="ExternalInput")
    xTs = nc.dram_tensor("xTs", [D, 512], bf16, kind="ExternalInput")
    w2T = nc.dram_tensor("w2T", [D, D], bf16, kind="ExternalInput")
    gvec = nc.dram_tensor("gvec", [P, 32], fp32, kind="ExternalInput")
    # gvec cols: 0-7 g1, 8-15 beta1, 16-23 g2, 24-31 beta2 (per 128-chunk)
    gb1 = nc.dram_tensor("gb1", [2, D], bf16, kind="ExternalInput")
    gb2 = nc.dram_tensor("gb2", [2, D], bf16, kind="ExternalInput")
    b2c = nc.dram_tensor("b2c", [P, 8], fp32, kind="ExternalInput")
    ones_p = nc.dram_tensor("ones_p", [P, 1], bf16, kind="ExternalInput")
    ones_r = nc.dram_tensor("ones_r", [1, 512], bf16, kind="ExternalInput")
    outT = nc.dram_tensor("outT", [D, 512], fp32, kind="ExternalOutput")

    with TileContext(nc) as tc:
        with tc.tile_pool(name="const", bufs=1) as cpool, \
             tc.tile_pool(name="big", bufs=1) as big, \
             tc.tile_pool(name="tmp", bufs=3) as tp, \
             tc.tile_pool(name="ps", bufs=1, space="PSUM") as ps:

            # HAM warmup during the input DMA wait (dense zero-matmuls)
            wzero = cpool.tile([P, 512], bf16, name="wzero")
            nc.vector.memset(wzero[:], 0.0)

            def warm(n):
                for i in range(n):
                    wp = ps.tile([P, 512], fp32, name="warm", tag="sa",
                                 bufs=2)
                    nc.tensor.matmul(wp[:], wzero[:, 0:P], wzero[:],
                                     start=True, stop=True)

            warm(25)

            yt = []
            for i in range(8):
                t = big.tile([P, 512], bf16, name=f"yt{i}", tag=f"yt{i}")
                nc.sync.dma_start(t[:], yT[i * P:(i + 1) * P, :])
                yt.append(t)
            onesp = cpool.tile([P, 1], bf16, name="onesp")
            nc.scalar.dma_start(onesp[:], ones_p[:])
            onesr = cpool.tile([1, 512], bf16, name="onesr")
            nc.scalar.dma_start(onesr[:], ones_r[:])
            gv = cpool.tile([P, 32], fp32, name="gv")
            nc.scalar.dma_start(gv[:], gvec[:])
            b2s = cpool.tile([P, 8], fp32, name="b2s")
            nc.scalar.dma_start(b2s[:], b2c[:])
            gb1_sb = cpool.tile([2, D], bf16, name="gb1_sb")
            nc.scalar.dma_start(gb1_sb[:], gb1[:])
            gb2_sb = cpool.tile([2, D], bf16, name="gb2_sb")
            nc.scalar.dma_start(gb2_sb[:], gb2[:])
            # nm tiles: row 1 = ones (written by DMA; engines can't start
            # a write at partition 1), row 0 = -mu*rstd written at LN time
            nm_t = {}
            for tag in ("a", "b"):
                t = cpool.tile([2, 512], bf16, name=f"nm{tag}")
                nc.scalar.dma_start(t[1:2, :], ones_r[:])
                nm_t[tag] = t
            w2sb = []
            for k in range(8):
                t = big.tile([P, D], bf16, name=f"w2sb{k}", tag=f"w2sb{k}")
                nc.scalar.dma_start(t[:], w2T[k * P:(k + 1) * P, :])
                w2sb.append(t)
            xts = []
            for i in range(8):
                t = big.tile([P, 512], bf16, name=f"xts{i}", tag=f"xts{i}")
                nc.sync.dma_start(t[:], xTs[i * P:(i + 1) * P, :])
                xts.append(t)

            def ln_stats(src, tag):
                """Accumulate sum and sum-of-squares; return (mu, rstd...)
                small tiles plus the rstd broadcast psum [128,512]."""
                psum_s = ps.tile([1, 512], fp32, name=f"ls{tag}", tag="sa",
                                 bufs=2)
                psum_q = ps.tile([1, 512], fp32, name=f"lq{tag}", tag="sb",
                                 bufs=2)
                for i in range(8):
                    nc.tensor.matmul(psum_s[:], onesp[:], src[i][:],
                                     start=(i == 0), stop=(i == 7))
                for i in range(8):
                    sq = tp.tile([P, 512], bf16, name="sq", tag="sq", bufs=2)
                    nc.scalar.activation(sq[:], src[i][:], AF.Square,
                                         scale=1.0)
                    nc.tensor.matmul(psum_q[:], onesp[:], sq[:],
                                     start=(i == 0), stop=(i == 7))
                return psum_s, psum_q

            def warm_f2(n):
                # keep-warm matmuls on the (currently idle) f2 psum bank so
                # the HAM clock gate stays released through the LN phase
                for i in range(n):
                    wp = ps.tile([P, 512], fp32, name="warmf", tag="f2ps0",
                                 bufs=1)
                    nc.tensor.matmul(wp[:], wzero[:, 0:P], wzero[:],
                                     start=True, stop=True)

            def layernorm(src, gb_sb, gcol, bcol, out_dt, tag,
                          stats=None):
                """LN over partition-dim features, 2 DVE passes per tile:
                t = src*rstd_bcast;  out = t*g[f] + (beta[f] - mu*rstd*g[f])
                with the second term a rank-2 PE broadcast."""
                psum_s, psum_q = stats if stats else ln_stats(src, tag)
                mu = tp.tile([1, 512], fp32, name=f"mu{tag}", tag="vec")
                nc.vector.tensor_scalar(mu[:], psum_s[:], 1.0 / D, None,
                                        ALU.mult)
                msq = tp.tile([1, 512], fp32, name=f"msq{tag}", tag="vec")
                nc.vector.tensor_scalar(msq[:], psum_q[:], 1.0 / D, None,
                                        ALU.mult)
                var = tp.tile([1, 512], fp32, name=f"var{tag}", tag="vec")
                nc.vector.tensor_tensor(var[:], mu[:], mu[:], ALU.mult)
                nc.vector.tensor_tensor(var[:], msq[:], var[:], ALU.subtract)
                nc.vector.tensor_scalar(var[:], var[:], EPS, None, ALU.add)
                sd = tp.tile([1, 512], fp32, name=f"sd{tag}", tag="vec")
                nc.scalar.activation(sd[:], var[:], AF.Sqrt, scale=1.0)
                rstd = tp.tile([1, 512], fp32, name=f"rstd{tag}", tag="vec")
                nc.vector.reciprocal(rstd[:], sd[:])
                rstd_b = tp.tile([1, 512], bf16, name=f"rb{tag}", tag="vecb",
                                 bufs=2)
                nc.vector.tensor_copy(rstd_b[:], rstd[:])
                nm = nm_t[tag]
                t0 = tp.tile([1, 512], fp32, name=f"t0{tag}", tag="vec")
                nc.vector.tensor_tensor(t0[:], mu[:], rstd[:], ALU.mult)
                nc.vector.tensor_scalar(nm[0:1, :], t0[:], -1.0, None,
                                        ALU.mult)
                pr = ps.tile([P, 512], fp32, name=f"pr{tag}", tag="sa",
                             bufs=2)
                nc.tensor.matmul(pr[:], onesr[0:1, 0:P], rstd_b[:],
                                 start=True, stop=True)
                outs = []
                for i in range(8):
                    isl = slice(i * P, (i + 1) * P)
                    pb = ps.tile([P, 512], fp32, name=f"pb{tag}", tag="sb",
                                 bufs=2)
                    nc.tensor.matmul(pb[:], gb_sb[:, isl], nm[:],
                                     start=True, stop=True)
                    if tag == "a":
                        warm_f2(3)
                    t = tp.tile([P, 512], bf16, name="lnt", tag="lnt",
                                bufs=2)
                    nc.vector.tensor_tensor(t[:], src[i][:], pr[:], ALU.mult)
                    o = tp.tile([P, 512], out_dt, name=f"lno{tag}",
                                tag=f"lno{tag}{i}", bufs=1)
                    nc.vector.affine_then_add(o[:], t[:], pb[:],
                                              gv[:, gcol + i:gcol + i + 1],
                                              0.0)
                    outs.append(o)
                return outs

            yln = layernorm(yt, gb1_sb, 0, 8, bf16, "a")

            # f2 (+ b2 + residual); k-outer so it starts when yln[0] lands.
            # LN2 stats accumulate inline as each t2r chunk is produced.
            t2r = [None] * 8
            psum_s2 = ps.tile([1, 512], fp32, name="lsb", tag="sa", bufs=2)
            psum_q2 = ps.tile([1, 512], fp32, name="lqb", tag="sb", bufs=2)
            for half in range(2):
                fcs = [4 * half + j for j in range(4)]
                pts = {fc: ps.tile([P, 512], fp32, name=f"f2ps{fc}",
                                   tag=f"f2ps{fc % 4}", bufs=1) for fc in fcs}
                for k in range(8):
                    for fc in fcs:
                        nc.tensor.matmul(pts[fc][:],
                                         w2sb[k][:, fc * P:(fc + 1) * P],
                                         yln[k][:],
                                         start=(k == 0), stop=(k == 7))
                for fc in fcs:
                    t = big.tile([P, 512], bf16, name=f"t2r{fc}",
                                 tag=f"t2r{fc}")
                    nc.vector.affine_then_add(t[:], pts[fc][:], xts[fc][:],
                                              1.0, b2s[:, fc:fc + 1])
                    t2r[fc] = t
                    nc.tensor.matmul(psum_s2[:], onesp[:], t[:],
                                     start=(fc == 0), stop=(fc == 7))
                    sq = tp.tile([P, 512], bf16, name="sq", tag="sq", bufs=2)
                    nc.scalar.activation(sq[:], t[:], AF.Square, scale=1.0)
                    nc.tensor.matmul(psum_q2[:], onesp[:], sq[:],
                                     start=(fc == 0), stop=(fc == 7))

            out_f = layernorm(t2r, gb2_sb, 16, 24, fp32, "b",
                              stats=(psum_s2, psum_q2))
            for i in range(8):
                nc.sync.dma_start(outT[i * P:(i + 1) * P, :], out_f[i][:])
    return nc


# ---------------------------------------------------------------- host side
def _plan_from_mask(mask):
    """Per (g): list of live kt tiles with live col range [off, end).

    Verifies every partially-masked 128x128 block matches the Toeplitz
    pattern keep(kpos<=qpos+c) the strip encoding can express.
    """
    keep = (mask.reshape(S, S) >= 0)
    plan = {}
    jmin, jmax = 10 ** 9, -10 ** 9
    partial = []  # (kt, qb) blocks that need strip masking
    for g in range(NG):
        kts = []
        for kt in range(NB):
            qbs = []
            for j in range(4):
                qb = 4 * g + j
                sub = keep[qb * P:(qb + 1) * P, kt * P:(kt + 1) * P]
                if sub.any():
                    qbs.append(j)
                    if not sub.all():
                        partial.append((kt, qb))
            if not qbs:
                continue
            lead, last = qbs[0], qbs[-1]
            # no fully-masked holes in the middle (true for causal)
            assert qbs == list(range(lead, last + 1)), "non-contiguous mask"
            off, end = lead * P, (last + 1) * P
            jj_lo = 4 * g - kt + 15 + lead
            jj_hi = 4 * g - kt + 15 + last
            jmin, jmax = min(jmin, jj_lo), max(jmax, jj_hi)
            kts.append((kt, off, end))
        plan[g] = kts
    if jmin > jmax:
        jmin, jmax = 15, 15
    # verify partial blocks match the Toeplitz encoding
    for kt, qb in partial:
        sub = keep[qb * P:(qb + 1) * P, kt * P:(kt + 1) * P]  # [qf, kf]
        jj = qb - kt + 15
        qf = np.arange(P)[:, None]
        kf = np.arange(P)[None, :]
        expect = (kf - qf) <= P * (jj - 15)
        assert (sub == expect).all(), "mask is not causal-Toeplitz"
    return plan, jmin, jmax


def _units_from_plan(plan):
    """Group consecutive full (kt, kt+1) tiles into pair units.

    Pair units share one [128,1024] psum / att tile and one bias window
    from strip2. Returns (units, pair_jjs)."""
    units = {}
    pair_jjs = set()
    for g in range(NG):
        kts = plan[g]
        full = {kt for kt, off, end in kts if off == 0 and end == 512}
        us = []
        skip = set()
        for kt, off, end in kts:
            if kt in skip:
                continue
            if (kt in full and kt % 2 == 0 and (kt + 1) in full):
                us.append(("pair", kt))
                skip.add(kt + 1)
                pair_jjs.add(4 * g - kt + 15)
            else:
                us.append(("single", kt, off, end))
        # av for the first unit initializes the full psum width
        if us:
            assert us[0][0] == "pair" or (us[0][2] == 0 and us[0][3] == 512)
        units[g] = us
    return units, sorted(pair_jjs)


def _build_strip(pos_w, jmin, n_strip):
    strip = np.zeros((P, n_strip * P), np.float32)
    pidx = np.arange(P)[:, None]
    fidx = np.arange(P)[None, :]
    for i in range(n_strip):
        jj = jmin + i
        base = M - 1 - P * (jj - 15)
        tile = pos_w[base + pidx - fidx].copy()
        # masked where kpos > qpos  <=>  p - f > 128*(jj-15)
        tile[(pidx - fidx) > P * (jj - 15)] = NEG
        strip[:, i * P:(i + 1) * P] = tile
    return strip.astype(bfdt)


def _build_strip2(strip, jmin, pair_jjs):
    """Pre-concatenated 1024-wide bias windows for pair units: window of
    kt (jj0) then window of kt+1 (jj0-1)."""
    n_pj = max(len(pair_jjs), 1)
    strip2 = np.zeros((P, n_pj * 1024), bfdt)
    for i, jj0 in enumerate(pair_jjs):
        w0 = (jj0 - jmin) * P
        w1 = (jj0 - 1 - jmin) * P
        strip2[:, i * 1024:i * 1024 + 512] = strip[:, w0:w0 + 512]
        strip2[:, i * 1024 + 512:(i + 1) * 1024] = strip[:, w1:w1 + 512]
    return strip2


def _get_compiled(mask_bytes, mask):
    if mask_bytes in _CACHE:
        return _CACHE[mask_bytes]
    plan, jmin, jmax = _plan_from_mask(mask)
    units, pair_jjs = _units_from_plan(plan)
    n_strip = jmax - jmin + 1
    nca = build_kernel_a(units, jmin, n_strip, pair_jjs)
    nca.compile()
    ncb = build_kernel_b()
    ncb.compile()
    _CACHE[mask_bytes] = (nca, ncb, jmin, n_strip, pair_jjs)
    return _CACHE[mask_bytes]


def kernel(x, mask, w1, b1, w2, b2, g1, beta1, g2, beta2, pos_w):
    x = np.asarray(x, np.float32)
    w1 = np.asarray(w1, np.float32)
    b1 = np.asarray(b1, np.float32)
    w2 = np.asarray(w2, np.float32)
    b2 = np.asarray(b2, np.float32)
    g1 = np.asarray(g1, np.float32)
    beta1 = np.asarray(beta1, np.float32)
    g2 = np.asarray(g2, np.float32)
    beta2 = np.asarray(beta2, np.float32)
    pos_w = np.asarray(pos_w, np.float32)
    mask_np = np.asarray(mask)

    nca, ncb, jmin, n_strip, pair_jjs = _get_compiled(mask_np.tobytes(),
                                                      mask_np)

    trace = bool(int(os.environ.get("HSTU_TRACE", "0")))
    strip = _build_strip(pos_w, jmin, n_strip)
    strip2 = _build_strip2(strip, jmin, pair_jjs)
    inv128 = np.full((P, P), 1.0 / P, bfdt)

    xT = [np.ascontiguousarray(x[b].T.astype(bfdt)) for b in range(B)]
    in_maps_a = []
    for c in range(8):
        b, hg = divmod(c, 4)
        heads = [4 * hg + i for i in range(4)]
        rows_q = np.concatenate([np.arange(D + h * HD, D + (h + 1) * HD)
                                 for h in heads])
        rows_k = np.concatenate([np.arange(2 * D + h * HD, 2 * D + (h + 1) * HD)
                                 for h in heads])
        rows_u = np.concatenate([np.arange(h * HD, (h + 1) * HD) for h in heads])
        rows_v = np.concatenate([np.arange(3 * D + h * HD, 3 * D + (h + 1) * HD)
                                 for h in heads])
        rows_qku = np.concatenate([rows_q, rows_k, rows_u])
        b1_2d = np.ascontiguousarray(b1[rows_qku].reshape(6, P).T)
        b1v = b1[rows_v]
        in_maps_a.append(dict(
            xT=xT[b],
            w1T_qku=np.ascontiguousarray(w1[rows_qku].T.astype(bfdt)),
            w1T_v=np.ascontiguousarray(w1[rows_v].T.astype(bfdt)),
            b1_2d=b1_2d,
            b1v2=np.ascontiguousarray(
                np.broadcast_to(np.tile(b1v, 2)[None, :], (P, 512))
            ).astype(bfdt),
            inv128=inv128, strip=strip, strip2=strip2,
        ))
    res_a = run_bass_kernel_spmd(nca, in_maps_a, core_ids=list(range(8)),
                                 trace=trace)

    yT_full = [np.empty((D, S), bfdt) for _ in range(B)]
    for c in range(8):
        b, hg = divmod(c, 4)
        yT_full[b][hg * DC:(hg + 1) * DC] = res_a.results[c]["yT_out"]

    w2T = np.ascontiguousarray(w2.T.astype(bfdt))
    b2c = np.ascontiguousarray(b2.reshape(8, P).T)
    gvec = np.concatenate([g1.reshape(8, P).T, beta1.reshape(8, P).T,
                           g2.reshape(8, P).T, beta2.reshape(8, P).T], axis=1)
    gvec = np.ascontiguousarray(gvec)
    gb1 = np.ascontiguousarray(np.stack([g1, beta1]).astype(bfdt))
    gb2 = np.ascontiguousarray(np.stack([g2, beta2]).astype(bfdt))
    ones_p = np.ones((P, 1), bfdt)
    ones_r = np.ones((1, 512), bfdt)
    in_maps_b = []
    for c in range(8):
        b, qc = divmod(c, 4)
        sl = slice(qc * 512, (qc + 1) * 512)
        in_maps_b.append(dict(
            yT=np.ascontiguousarray(yT_full[b][:, sl]),
            xTs=np.ascontiguousarray(xT[b][:, sl]),
            w2T=w2T, b2c=b2c, gvec=gvec, gb1=gb1, gb2=gb2,
            ones_p=ones_p, ones_r=ones_r,
        ))
    res_b = run_bass_kernel_spmd(ncb, in_maps_b, core_ids=list(range(8)),
                                 trace=trace)

    out = np.empty((B, S, D), np.float32)
    for c in range(8):
        b, qc = divmod(c, 4)
        out[b, qc * 512:(qc + 1) * 512] = res_b.results[c]["outT"].T
    kernel.last_results = (res_a, res_b)
    return out


# revision 36
# speedup vs baseline: 1.1220x; 1.1220x over previous
"""HSTU block kernel for 8 TRN2 NeuronCores (Bass/Tile, bf16 matmuls).

Sharding: phase 1 (f1 + attention + u-gating) is data-parallel over batch
(B=2) x tensor-parallel over head groups (4 heads/core). Phase 2
(ln1 -> f2 -> +residual -> ln2) is row-parallel (512 rows/core). Host
gathers/reshards between the two launches; everything on-device is
feature-major so no transposes are ever needed.

v2: bf16 data path, causal mask folded into the pos-bias strip (-60 at
masked entries, silu(-60) == 0), attention bias adds on DVE instead of
PE, head-pair tiles via matmul tile_position, and a software-pipelined
emit order (qk block g interleaved with av block g-1, f1 slab g+1
between them) so the PE never head-of-line blocks on scalar silu.
"""
import os
import numpy as np
import ml_dtypes

import concourse.bacc as bacc
import concourse.mybir as mybir
from concourse.tile import TileContext
from concourse.bass_utils import run_bass_kernel_spmd

fp32 = mybir.dt.float32
bf16 = mybir.dt.bfloat16
AF = mybir.ActivationFunctionType
ALU = mybir.AluOpType

B, S, D, H, M = 2, 2048, 1024, 16, 4096
HD = D // H          # 64
EPS = 1e-5
P = 128
NB = S // P          # 16 seq blocks of 128
NG = S // 512        # 4 q-groups of 512
DC = 4 * HD          # 256 features per core in phase 1
NEG = -60.0          # silu(-60) == 0 in fp32: masked-score sentinel

_CACHE = {}

bfdt = ml_dtypes.bfloat16


# ---------------------------------------------------------------- kernel A
def build_kernel_a(units, jmin, n_strip, pair_jjs):
    """units[g] = [("pair", kt)] / [("single", kt, off, end)] per q-group.

    pair units cover (kt, kt+1), both with full [0,512) live range; their
    bias windows are pre-concatenated in strip2 indexed by jj0=4g-kt+15.
    """
    nc = bacc.Bacc("TRN2", target_bir_lowering=False, debug=False, num_devices=8)

    xT = nc.dram_tensor("xT", [D, S], bf16, kind="ExternalInput")
    w1T_qku = nc.dram_tensor("w1T_qku", [D, 768], bf16, kind="ExternalInput")
    w1T_v = nc.dram_tensor("w1T_v", [D, DC], bf16, kind="ExternalInput")
    b1_2d = nc.dram_tensor("b1_2d", [P, 6], fp32, kind="ExternalInput")
    b1v2 = nc.dram_tensor("b1v2", [P, 512], bf16, kind="ExternalInput")
    inv128 = nc.dram_tensor("inv128", [P, P], bf16, kind="ExternalInput")
    strip = nc.dram_tensor("strip", [P, n_strip * P], bf16, kind="ExternalInput")
    n_pj = max(len(pair_jjs), 1)
    strip2 = nc.dram_tensor("strip2", [P, n_pj * 1024], bf16,
                            kind="ExternalInput")
    yT_out = nc.dram_tensor("yT_out", [DC, S], bf16, kind="ExternalOutput")
    pj_idx = {jj: i for i, jj in enumerate(pair_jjs)}

    with TileContext(nc) as tc:
        with tc.tile_pool(name="const", bufs=1) as cpool, \
             tc.tile_pool(name="wpool", bufs=2) as wpool, \
             tc.tile_pool(name="big", bufs=1) as big, \
             tc.tile_pool(name="att", bufs=1) as apool, \
             tc.tile_pool(name="out", bufs=2) as opool, \
             tc.tile_pool(name="ps", bufs=1, space="PSUM") as ps:

            # HAM warmup: dense zero-matmuls (full 128-row contraction) keep
            # the PE activity monitor busy during the initial DMA wait so
            # the clock gate is released before real matmuls start
            wzero = cpool.tile([P, 512], bf16, name="wzero")
            nc.vector.memset(wzero[:], 0.0)
            for i in range(30):
                wp = ps.tile([P, 512], fp32, name="warm", tag="mm", bufs=4)
                nc.tensor.matmul(wp[:], wzero[:, 0:P], wzero[:],
                                 start=True, stop=True)

            # critical path: x slab 0 (sync queue) and w1 (scalar queue)
            # land in parallel so the first matmul starts early
            xs0 = []
            wq = []
            for k in range(8):
                t = wpool.tile([P, 512], bf16, name="xs", tag=f"xs{k}", bufs=2)
                nc.sync.dma_start(t[:], xT[k * P:(k + 1) * P, 0:512])
                xs0.append(t)
                w = big.tile([P, 768], bf16, name=f"wq{k}", tag=f"wq{k}")
                nc.scalar.dma_start(w[:], w1T_qku[k * P:(k + 1) * P, :])
                wq.append(w)
            b1_sb = cpool.tile([P, 6], fp32, name="b1_sb")
            nc.scalar.dma_start(b1_sb[:], b1_2d[:])
            strip_sb = big.tile([P, n_strip * P], bf16, name="strip_sb",
                                tag="strip")
            nc.sync.dma_start(strip_sb[:], strip[:])
            strip2_sb = big.tile([P, n_pj * 1024], bf16, name="strip2_sb",
                                 tag="strip2")
            nc.sync.dma_start(strip2_sb[:], strip2[:])
            wv = []
            for k in range(8):
                t = big.tile([P, DC], bf16, name=f"w1v{k}", tag=f"w1v{k}")
                nc.sync.dma_start(t[:], w1T_v[k * P:(k + 1) * P, :])
                wv.append(t)
            b1v_sb = cpool.tile([P, 512], bf16, name="b1v_sb")
            nc.sync.dma_start(b1v_sb[:], b1v2[:])
            inv128_sb = cpool.tile([P, P], bf16, name="inv128_sb")
            nc.sync.dma_start(inv128_sb[:], inv128[:])

            # persistent: k heads (pair tiles) + v (sc-pair tiles)
            kh = [big.tile([P, S], bf16, name=f"khp{p}", tag=f"khp{p}")
                  for p in range(2)]
            v_sb = [big.tile([P, 512], bf16, name=f"v{scp}", tag=f"v{scp}")
                    for scp in range(8)]

            def f1_alloc():
                qs = [wpool.tile([P, 512], bf16, name="qs", tag=f"qs{i}",
                                 bufs=2) for i in range(2)]
                us = [wpool.tile([P, 512], bf16, name="us", tag=f"us{i}",
                                 bufs=3) for i in range(2)]
                return qs, us

            def f1_act(sg, fc, pt, qs, us):
                sl = slice(sg * 512, (sg + 1) * 512)
                bcol = b1_sb[:, fc:fc + 1]
                if fc < 2:
                    nc.scalar.activation(qs[fc][:], pt[:], AF.Silu,
                                         bias=bcol, scale=1.0)
                elif fc < 4:
                    nc.scalar.activation(kh[fc - 2][:, sl], pt[:],
                                         AF.Silu, bias=bcol, scale=1.0)
                else:
                    nc.scalar.activation(us[fc - 4][:], pt[:], AF.Silu,
                                         bias=bcol, scale=1.0)

            def f1_fc_unit(sg, fc, xs, qs, us):
                pt = ps.tile([P, 512], fp32, name="f1ps", tag="mm", bufs=4)
                for k in range(8):
                    nc.tensor.matmul(pt[:], wq[k][:, fc * P:(fc + 1) * P],
                                     xs[k][:], start=(k == 0), stop=(k == 7))
                f1_act(sg, fc, pt, qs, us)

            def f1_v_unit(sg, scp, xs):
                # v: natural (seq-major) layout, 2 sc per psum tile
                pt = ps.tile([P, 512], fp32, name="f1vps", tag="mm", bufs=4)
                for c in range(2):
                    csl = slice(c * DC, (c + 1) * DC)
                    xoff = (2 * (scp - 2 * sg) + c) * P
                    for k in range(8):
                        nc.tensor.matmul(
                            pt[:, csl], xs[k][:, xoff:xoff + P], wv[k][:],
                            start=(k == 0), stop=False)
                    nc.tensor.matmul(pt[:, csl], inv128_sb[:],
                                     b1v_sb[:, csl],
                                     start=False, stop=True)
                nc.scalar.activation(v_sb[scp][:], pt[:], AF.Silu,
                                     scale=1.0)

            def f1_slab0(xs):
                """slab 0, emitted standalone: k-outer over fc 0-3 so
                compute starts as soon as chunk 0 lands."""
                qs, us = f1_alloc()
                pts = [ps.tile([P, 512], fp32, name="f1ps", tag="mm",
                               bufs=4) for _ in range(4)]
                for k in range(8):
                    for fc in range(4):
                        nc.tensor.matmul(
                            pts[fc][:], wq[k][:, fc * P:(fc + 1) * P],
                            xs[k][:], start=(k == 0), stop=(k == 7))
                for fc in range(4):
                    f1_act(0, fc, pts[fc], qs, us)
                for fc in (4, 5):
                    f1_fc_unit(0, fc, xs, qs, us)
                for scp in (0, 1):
                    f1_v_unit(0, scp, xs)
                return qs, us

            def f1_thunks(sg, xs):
                """f1 for slab sg as unit thunks, interleaved into the
                attention stream to keep the PE dense (HAM stays warm)."""
                qs, us = f1_alloc()
                thunks = [lambda fc=fc: f1_fc_unit(sg, fc, xs, qs, us)
                          for fc in range(6)]
                thunks += [lambda scp=scp: f1_v_unit(sg, scp, xs)
                           for scp in (2 * sg, 2 * sg + 1)]
                return qs, us, thunks

            state = {}  # per g: (qs, us, att tiles, avp psums)

            def emit_qk(g, p, ui, unit):
                """scores + bias + silu for head pair p, one unit."""
                qs = state[g]["qs"]
                if unit[0] == "pair":
                    kt = unit[1]
                    jj0 = 4 * g - kt + 15
                    s2 = pj_idx[jj0] * 1024
                    for hp in range(2):
                        h0 = hp * 64
                        spp = ps.tile([P, 1024], fp32, name="sp2", tag="sp2",
                                      bufs=2)
                        for c in range(2):
                            nc.tensor.matmul(
                                spp[:, c * 512:(c + 1) * 512],
                                kh[p][h0:h0 + 64,
                                      (kt + c) * P:(kt + c + 1) * P],
                                qs[p][h0:h0 + 64, :],
                                start=True, stop=True)
                        att = apool.tile([P, 1024], bf16, name="att2",
                                         tag=f"a2_{p}{hp}u{ui}", bufs=1)
                        nc.vector.tensor_tensor(
                            att[:], spp[:], strip2_sb[:, s2:s2 + 1024],
                            ALU.add)
                        nc.scalar.activation(att[:], att[:], AF.Silu,
                                             scale=1.0)
                        state[g]["att"][(p, hp, ui)] = att
                else:
                    _, kt, off, end = unit
                    sb0 = (4 * g - kt + 15 - jmin) * P
                    for hp in range(2):
                        h0 = hp * 64
                        spp = ps.tile([P, 512], fp32, name="sps", tag="mm",
                                      bufs=4)
                        nc.tensor.matmul(
                            spp[:, off:end],
                            kh[p][h0:h0 + 64, kt * P:(kt + 1) * P],
                            qs[p][h0:h0 + 64, off:end],
                            start=True, stop=True)
                        att = apool.tile([P, 512], bf16, name="att",
                                         tag=f"a{p}{hp}u{ui}", bufs=1)
                        nc.vector.tensor_tensor(
                            att[:, off:end], spp[:, off:end],
                            strip_sb[:, sb0 + off:sb0 + end], ALU.add)
                        nc.scalar.activation(att[:, off:end],
                                             att[:, off:end],
                                             AF.Silu, scale=1.0)
                        state[g]["att"][(p, hp, ui)] = att

            def emit_av(g, p, ui, unit, n_units):
                """accumulation steps of att @ v for head pair p, one unit."""
                if ui == 0:
                    state[g]["avp"][p] = ps.tile([P, 512], fp32, name="avp",
                                                 tag=f"avp{p}", bufs=1)
                avp = state[g]["avp"][p]
                att_d = state[g]["att"]
                last = (ui == n_units - 1)
                if unit[0] == "pair":
                    kt0 = unit[1]
                    for hp in range(2):
                        h = 2 * p + hp
                        att = att_d[(p, hp, ui)]
                        for c in range(2):
                            kt = kt0 + c
                            scp, cc = kt // 2, kt % 2
                            nc.tensor.matmul(
                                avp[hp * 64:(hp + 1) * 64, :],
                                v_sb[scp][:, cc * DC + h * 64:
                                          cc * DC + (h + 1) * 64],
                                att[:, c * 512:(c + 1) * 512],
                                start=(ui == 0 and c == 0),
                                stop=(last and c == 1))
                else:
                    _, kt, off, end = unit
                    scp, cc = kt // 2, kt % 2
                    ao, ae = (0, 512) if ui == 0 else (off, end)
                    for hp in range(2):
                        h = 2 * p + hp
                        att = att_d[(p, hp, ui)]
                        nc.tensor.matmul(
                            avp[hp * 64:(hp + 1) * 64, ao:ae],
                            v_sb[scp][:, cc * DC + h * 64:
                                      cc * DC + (h + 1) * 64],
                            att[:, ao:ae],
                            start=(ui == 0), stop=last)

            def emit_yg(g):
                """u-gating + output DMA for group g (both pairs)."""
                us = state[g]["us"]
                for p in range(2):
                    avp = state[g]["avp"][p]
                    yg = opool.tile([P, 512], bf16, name="yg", tag=f"yg{p}",
                                    bufs=2)
                    nc.vector.tensor_tensor(yg[:], avp[:], us[p][:], ALU.mult)
                    nc.sync.dma_start(
                        yT_out[p * P:(p + 1) * P, g * 512:(g + 1) * 512],
                        yg[:])
                del state[g]

            def prefetch_xs(sg):
                xs_n = []
                nsl = slice(sg * 512, (sg + 1) * 512)
                for k in range(8):
                    t = wpool.tile([P, 512], bf16, name="xs",
                                   tag=f"xs{k}", bufs=2)
                    nc.sync.dma_start(t[:], xT[k * P:(k + 1) * P, nsl])
                    xs_n.append(t)
                return xs_n

            xs_pref = {}

            def block(g):
                """qk stream for g, av stream for g-1, and f1 units for
                slab g+1, all interleaved to keep every engine streaming."""
                if g < NG - 1:
                    qs_n, us_n, thunks = f1_thunks(g + 1, xs_pref[g + 1])
                    state[g + 1] = {"qs": qs_n, "us": us_n, "att": {},
                                    "avp": {}}
                else:
                    thunks = []
                if g < NG - 2:
                    xs_pref[g + 2] = prefetch_xs(g + 2)
                prev = units[g - 1] if g > 0 else []
                cur = units[g]
                att_stream = [(p, ui, unit) for p in range(2)
                              for ui, unit in enumerate(cur)]
                stride = max(1, -(-len(att_stream) // (len(thunks) + 1)))
                ti = 0
                n_av = 0
                for idx, (p, ui, unit) in enumerate(att_stream):
                    if idx % stride == 0 and ti < len(thunks):
                        thunks[ti]()
                        ti += 1
                    if ui < len(prev):
                        emit_av(g - 1, p, ui, prev[ui], len(prev))
                        n_av += 1
                        if n_av == 2 * len(prev):
                            emit_yg(g - 1)
                    emit_qk(g, p, ui, unit)
                while ti < len(thunks):
                    thunks[ti]()
                    ti += 1

            qs, us = f1_slab0(xs0)
            state[0] = {"qs": qs, "us": us, "att": {}, "avp": {}}
            if NG > 1:
                xs_pref[1] = prefetch_xs(1)
            for g in range(NG):
                block(g)
            # drain: av + yg for the last group
            gl = NG - 1
            cur = units[gl]
            for p in range(2):
                for ui, unit in enumerate(cur):
                    emit_av(gl, p, ui, unit, len(cur))
            emit_yg(gl)
    return nc


# ---------------------------------------------------------------- kernel B
def build_kernel_b():
    nc = bacc.Bacc("TRN2", target_bir_lowering=False, debug=False, num_devices=8)

    yT = nc.dram_tensor("yT", [D, 512], bf16, kind="ExternalInput")
    xTs = nc.dram_tensor("xTs", [D, 512], bf16, kind="ExternalInput")
    w2T = nc.dram_tensor("w2T", [D, D], bf16, kind="ExternalInput")
    gvec = nc.dram_tensor("gvec", [P, 32], fp32, kind="ExternalInput")
    # gvec cols: 0-7 g1, 8-15 beta1, 16-23 g2, 24-31 beta2 (per 128-chunk)
    gb1 = nc.dram_tensor("gb1", [2, D], bf16, kind="ExternalInput")
    gb2 = nc.dram_tensor("gb2", [2, D], bf16, kind="ExternalInput")
    b2c = nc.dram_tensor("b2c", [P, 8], fp32, kind="ExternalInput")
    ones_p = nc.dram_tensor("ones_p", [P, 1], bf16, kind="ExternalInput")
    ones_r = nc.dram_tensor("ones_r", [1, 512], bf16, kind="ExternalInput")
    outT = nc.dram_tensor("outT", [D, 512], fp32, kind="ExternalOutput")

    with TileContext(nc) as tc:
        with tc.tile_pool(name="const", bufs=1) as cpool, \
             tc.tile_pool(name="big", bufs=1) as big, \
             tc.tile_pool(name="tmp", bufs=3) as tp, \
             tc.tile_pool(name="ps", bufs=1, space="PSUM") as ps:

            yt = []
            for i in range(8):
                t = big.tile([P, 512], bf16, name=f"yt{i}", tag=f"yt{i}")
                nc.sync.dma_start(t[:], yT[i * P:(i + 1) * P, :])
                yt.append(t)
            onesp = cpool.tile([P, 1], bf16, name="onesp")
            nc.scalar.dma_start(onesp[:], ones_p[:])
            onesr = cpool.tile([1, 512], bf16, name="onesr")
            nc.scalar.dma_start(onesr[:], ones_r[:])
            gv = cpool.tile([P, 32], fp32, name="gv")
            nc.scalar.dma_start(gv[:], gvec[:])
            b2s = cpool.tile([P, 8], fp32, name="b2s")
            nc.scalar.dma_start(b2s[:], b2c[:])
            gb1_sb = cpool.tile([2, D], bf16, name="gb1_sb")
            nc.scalar.dma_start(gb1_sb[:], gb1[:])
            gb2_sb = cpool.tile([2, D], bf16, name="gb2_sb")
            nc.scalar.dma_start(gb2_sb[:], gb2[:])
            # nm tiles: row 1 = ones (written by DMA; engines can't start
            # a write at partition 1), row 0 = -mu*rstd written at LN time
            nm_t = {}
            for tag in ("a", "b"):
                t = cpool.tile([2, 512], bf16, name=f"nm{tag}")
                nc.scalar.dma_start(t[1:2, :], ones_r[:])
                nm_t[tag] = t
            w2sb = []
            for k in range(8):
                t = big.tile([P, D], bf16, name=f"w2sb{k}", tag=f"w2sb{k}")
                nc.scalar.dma_start(t[:], w2T[k * P:(k + 1) * P, :])
                w2sb.append(t)
            xts = []
            for i in range(8):
                t = big.tile([P, 512], bf16, name=f"xts{i}", tag=f"xts{i}")
                nc.sync.dma_start(t[:], xTs[i * P:(i + 1) * P, :])
                xts.append(t)

            def ln_stats(src, tag):
                """Accumulate sum and sum-of-squares; return (mu, rstd...)
                small tiles plus the rstd broadcast psum [128,512]."""
                psum_s = ps.tile([1, 512], fp32, name=f"ls{tag}", tag="sa",
                                 bufs=2)
                psum_q = ps.tile([1, 512], fp32, name=f"lq{tag}", tag="sb",
                                 bufs=2)
                for i in range(8):
                    nc.tensor.matmul(psum_s[:], onesp[:], src[i][:],
                                     start=(i == 0), stop=(i == 7))
                for i in range(8):
                    sq = tp.tile([P, 512], bf16, name="sq", tag="sq", bufs=2)
                    nc.scalar.activation(sq[:], src[i][:], AF.Square,
                                         scale=1.0)
                    nc.tensor.matmul(psum_q[:], onesp[:], sq[:],
                                     start=(i == 0), stop=(i == 7))
                return psum_s, psum_q

            def layernorm(src, gb_sb, gcol, bcol, out_dt, tag,
                          stats=None):
                """LN over partition-dim features, 2 DVE passes per tile:
                t = src*rstd_bcast;  out = t*g[f] + (beta[f] - mu*rstd*g[f])
                with the second term a rank-2 PE broadcast."""
                psum_s, psum_q = stats if stats else ln_stats(src, tag)
                mu = tp.tile([1, 512], fp32, name=f"mu{tag}", tag="vec")
                nc.vector.tensor_scalar(mu[:], psum_s[:], 1.0 / D, None,
                                        ALU.mult)
                msq = tp.tile([1, 512], fp32, name=f"msq{tag}", tag="vec")
                nc.vector.tensor_scalar(msq[:], psum_q[:], 1.0 / D, None,
                                        ALU.mult)
                var = tp.tile([1, 512], fp32, name=f"var{tag}", tag="vec")
                nc.vector.tensor_tensor(var[:], mu[:], mu[:], ALU.mult)
                nc.vector.tensor_tensor(var[:], msq[:], var[:], ALU.subtract)
                nc.vector.tensor_scalar(var[:], var[:], EPS, None, ALU.add)
                sd = tp.tile([1, 512], fp32, name=f"sd{tag}", tag="vec")
                nc.scalar.activation(sd[:], var[:], AF.Sqrt, scale=1.0)
                rstd = tp.tile([1, 512], fp32, name=f"rstd{tag}", tag="vec")
                nc.vector.reciprocal(rstd[:], sd[:])
                rstd_b = tp.tile([1, 512], bf16, name=f"rb{tag}", tag="vecb",
                                 bufs=2)
                nc.vector.tensor_copy(rstd_b[:], rstd[:])
                nm = nm_t[tag]
                t0 = tp.tile([1, 512], fp32, name=f"t0{tag}", tag="vec")
                nc.vector.tensor_tensor(t0[:], mu[:], rstd[:], ALU.mult)
                nc.vector.tensor_scalar(nm[0:1, :], t0[:], -1.0, None,
                                        ALU.mult)
                pr = ps.tile([P, 512], fp32, name=f"pr{tag}", tag="sa",
                             bufs=2)
                nc.tensor.matmul(pr[:], onesr[0:1, 0:P], rstd_b[:],
                                 start=True, stop=True)
                outs = []
                for i in range(8):
                    isl = slice(i * P, (i + 1) * P)
                    pb = ps.tile([P, 512], fp32, name=f"pb{tag}", tag="sb",
                                 bufs=2)
                    nc.tensor.matmul(pb[:], gb_sb[:, isl], nm[:],
                                     start=True, stop=True)
                    t = tp.tile([P, 512], bf16, name="lnt", tag="lnt",
                                bufs=2)
                    nc.vector.tensor_tensor(t[:], src[i][:], pr[:], ALU.mult)
                    o = tp.tile([P, 512], out_dt, name=f"lno{tag}",
                                tag=f"lno{tag}{i}", bufs=1)
                    nc.vector.affine_then_add(o[:], t[:], pb[:],
                                              gv[:, gcol + i:gcol + i + 1],
                                              0.0)
                    outs.append(o)
                return outs

            yln = layernorm(yt, gb1_sb, 0, 8, bf16, "a")

            # f2 (+ b2 + residual); k-outer so it starts when yln[0] lands.
            # LN2 stats accumulate inline as each t2r chunk is produced.
            t2r = [None] * 8
            psum_s2 = ps.tile([1, 512], fp32, name="lsb", tag="sa", bufs=2)
            psum_q2 = ps.tile([1, 512], fp32, name="lqb", tag="sb", bufs=2)
            for half in range(2):
                fcs = [4 * half + j for j in range(4)]
                pts = {fc: ps.tile([P, 512], fp32, name=f"f2ps{fc}",
                                   tag=f"f2ps{fc % 4}", bufs=1) for fc in fcs}
                for k in range(8):
                    for fc in fcs:
                        nc.tensor.matmul(pts[fc][:],
                                         w2sb[k][:, fc * P:(fc + 1) * P],
                                         yln[k][:],
                                         start=(k == 0), stop=(k == 7))
                for fc in fcs:
                    t = big.tile([P, 512], bf16, name=f"t2r{fc}",
                                 tag=f"t2r{fc}")
                    nc.vector.affine_then_add(t[:], pts[fc][:], xts[fc][:],
                                              1.0, b2s[:, fc:fc + 1])
                    t2r[fc] = t
                    nc.tensor.matmul(psum_s2[:], onesp[:], t[:],
                                     start=(fc == 0), stop=(fc == 7))
                    sq = tp.tile([P, 512], bf16, name="sq", tag="sq", bufs=2)
                    nc.scalar.activation(sq[:], t[:], AF.Square, scale=1.0)
                    nc.tensor.matmul(psum_q2[:], onesp[:], sq[:],
                                     start=(fc == 0), stop=(fc == 7))

            out_f = layernorm(t2r, gb2_sb, 16, 24, fp32, "b",
                              stats=(psum_s2, psum_q2))
            for i in range(8):
                nc.sync.dma_start(outT[i * P:(i + 1) * P, :], out_f[i][:])
    return nc


# ---------------------------------------------------------------- host side
def _plan_from_mask(mask):
    """Per (g): list of live kt tiles with live col range [off, end).

    Verifies every partially-masked 128x128 block matches the Toeplitz
    pattern keep(kpos<=qpos+c) the strip encoding can express.
    """
    keep = (mask.reshape(S, S) >= 0)
    plan = {}
    jmin, jmax = 10 ** 9, -10 ** 9
    partial = []  # (kt, qb) blocks that need strip masking
    for g in range(NG):
        kts = []
        for kt in range(NB):
            qbs = []
            for j in range(4):
                qb = 4 * g + j
                sub = keep[qb * P:(qb + 1) * P, kt * P:(kt + 1) * P]
                if sub.any():
                    qbs.append(j)
                    if not sub.all():
                        partial.append((kt, qb))
            if not qbs:
                continue
            lead, last = qbs[0], qbs[-1]
            # no fully-masked holes in the middle (true for causal)
            assert qbs == list(range(lead, last + 1)), "non-contiguous mask"
            off, end = lead * P, (last + 1) * P
            jj_lo = 4 * g - kt + 15 + lead
            jj_hi = 4 * g - kt + 15 + last
            jmin, jmax = min(jmin, jj_lo), max(jmax, jj_hi)
            kts.append((kt, off, end))
        plan[g] = kts
    if jmin > jmax:
        jmin, jmax = 15, 15
    # verify partial blocks match the Toeplitz encoding
    for kt, qb in partial:
        sub = keep[qb * P:(qb + 1) * P, kt * P:(kt + 1) * P]  # [qf, kf]
        jj = qb - kt + 15
        qf = np.arange(P)[:, None]
        kf = np.arange(P)[None, :]
        expect = (kf - qf) <= P * (jj - 15)
        assert (sub == expect).all(), "mask is not causal-Toeplitz"
    return plan, jmin, jmax


def _units_from_plan(plan):
    """Group consecutive full (kt, kt+1) tiles into pair units.

    Pair units share one [128,1024] psum / att tile and one bias window
    from strip2. Returns (units, pair_jjs)."""
    units = {}
    pair_jjs = set()
    for g in range(NG):
        kts = plan[g]
        full = {kt for kt, off, end in kts if off == 0 and end == 512}
        us = []
        skip = set()
        for kt, off, end in kts:
            if kt in skip:
                continue
            # pairing measured neutral on PE and worse for HAM; disabled
            if False and (kt in full and kt % 2 == 0 and (kt + 1) in full):
                us.append(("pair", kt))
                skip.add(kt + 1)
                pair_jjs.add(4 * g - kt + 15)
            else:
                us.append(("single", kt, off, end))
        # av for the first unit initializes the full psum width
        if us:
            assert us[0][0] == "pair" or (us[0][2] == 0 and us[0][3] == 512)
        units[g] = us
    return units, sorted(pair_jjs)


def _build_strip(pos_w, jmin, n_strip):
    strip = np.zeros((P, n_strip * P), np.float32)
    pidx = np.arange(P)[:, None]
    fidx = np.arange(P)[None, :]
    for i in range(n_strip):
        jj = jmin + i
        base = M - 1 - P * (jj - 15)
        tile = pos_w[base + pidx - fidx].copy()
        # masked where kpos > qpos  <=>  p - f > 128*(jj-15)
        tile[(pidx - fidx) > P * (jj - 15)] = NEG
        strip[:, i * P:(i + 1) * P] = tile
    return strip.astype(bfdt)


def _build_strip2(strip, jmin, pair_jjs):
    """Pre-concatenated 1024-wide bias windows for pair units: window of
    kt (jj0) then window of kt+1 (jj0-1)."""
    n_pj = max(len(pair_jjs), 1)
    strip2 = np.zeros((P, n_pj * 1024), bfdt)
    for i, jj0 in enumerate(pair_jjs):
        w0 = (jj0 - jmin) * P
        w1 = (jj0 - 1 - jmin) * P
        strip2[:, i * 1024:i * 1024 + 512] = strip[:, w0:w0 + 512]
        strip2[:, i * 1024 + 512:(i + 1) * 1024] = strip[:, w1:w1 + 512]
    return strip2


def _get_compiled(mask_bytes, mask):
    if mask_bytes in _CACHE:
        return _CACHE[mask_bytes]
    plan, jmin, jmax = _plan_from_mask(mask)
    units, pair_jjs = _units_from_plan(plan)
    n_strip = jmax - jmin + 1
    nca = build_kernel_a(units, jmin, n_strip, pair_jjs)
    nca.compile()
    ncb = build_kernel_b()
    ncb.compile()
    _CACHE[mask_bytes] = (nca, ncb, jmin, n_strip, pair_jjs)
    return _CACHE[mask_bytes]


def kernel(x, mask, w1, b1, w2, b2, g1, beta1, g2, beta2, pos_w):
    x = np.asarray(x, np.float32)
    w1 = np.asarray(w1, np.float32)
    b1 = np.asarray(b1, np.float32)
    w2 = np.asarray(w2, np.float32)
    b2 = np.asarray(b2, np.float32)
    g1 = np.asarray(g1, np.float32)
    beta1 = np.asarray(beta1, np.float32)
    g2 = np.asarray(g2, np.float32)
    beta2 = np.asarray(beta2, np.float32)
    pos_w = np.asarray(pos_w, np.float32)
    mask_np = np.asarray(mask)

    nca, ncb, jmin, n_strip, pair_jjs = _get_compiled(mask_np.tobytes(),
                                                      mask_np)

    trace = bool(int(os.environ.get("HSTU_TRACE", "0")))
    strip = _build_strip(pos_w, jmin, n_strip)
    strip2 = _build_strip2(strip, jmin, pair_jjs)
    inv128 = np.full((P, P), 1.0 / P, bfdt)

    xT = [np.ascontiguousarray(x[b].T.astype(bfdt)) for b in range(B)]
    in_maps_a = []
    for c in range(8):
        b, hg = divmod(c, 4)
        heads = [4 * hg + i for i in range(4)]
        rows_q = np.concatenate([np.arange(D + h * HD, D + (h + 1) * HD)
                                 for h in heads])
        rows_k = np.concatenate([np.arange(2 * D + h * HD, 2 * D + (h + 1) * HD)
                                 for h in heads])
        rows_u = np.concatenate([np.arange(h * HD, (h + 1) * HD) for h in heads])
        rows_v = np.concatenate([np.arange(3 * D + h * HD, 3 * D + (h + 1) * HD)
                                 for h in heads])
        rows_qku = np.concatenate([rows_q, rows_k, rows_u])
        b1_2d = np.ascontiguousarray(b1[rows_qku].reshape(6, P).T)
        b1v = b1[rows_v]
        in_maps_a.append(dict(
            xT=xT[b],
            w1T_qku=np.ascontiguousarray(w1[rows_qku].T.astype(bfdt)),
            w1T_v=np.ascontiguousarray(w1[rows_v].T.astype(bfdt)),
            b1_2d=b1_2d,
            b1v2=np.ascontiguousarray(
                np.broadcast_to(np.tile(b1v, 2)[None, :], (P, 512))
            ).astype(bfdt),
            inv128=inv128, strip=strip, strip2=strip2,
        ))
    res_a = run_bass_kernel_spmd(nca, in_maps_a, core_ids=list(range(8)),
                                 trace=trace)

    yT_full = [np.empty((D, S), bfdt) for _ in range(B)]
    for c in range(8):
        b, hg = divmod(c, 4)
        yT_full[b][hg * DC:(hg + 1) * DC] = res_a.results[c]["yT_out"]

    w2T = np.ascontiguousarray(w2.T.astype(bfdt))
    b2c = np.ascontiguousarray(b2.reshape(8, P).T)
    gvec = np.concatenate([g1.reshape(8, P).T, beta1.reshape(8, P).T,
                           g2.reshape(8, P).T, beta2.reshape(8, P).T], axis=1)
    gvec = np.ascontiguousarray(gvec)
    gb1 = np.ascontiguousarray(np.stack([g1, beta1]).astype(bfdt))
    gb2 = np.ascontiguousarray(np.stack([g2, beta2]).astype(bfdt))
    ones_p = np.ones((P, 1), bfdt)
    ones_r = np.ones((1, 512), bfdt)
    in_maps_b = []
    for c in range(8):
        b, qc = divmod(c, 4)
        sl = slice(qc * 512, (qc + 1) * 512)
        in_maps_b.append(dict(
            yT=np.ascontiguousarray(yT_full[b][:, sl]),
            xTs=np.ascontiguousarray(xT[b][:, sl]),
            w2T=w2T, b2c=b2c, gvec=gvec, gb1=gb1, gb2=gb2,
            ones_p=ones_p, ones_r=ones_r,
        ))
    res_b = run_bass_kernel_spmd(ncb, in_maps_b, core_ids=list(range(8)),
                                 trace=trace)

    out = np.empty((B, S, D), np.float32)
    for c in range(8):
        b, qc = divmod(c, 4)
        out[b, qc * 512:(qc + 1) * 512] = res_b.results[c]["outT"].T
    kernel.last_results = (res_a, res_b)
    return out
